# revision 18
# baseline (speedup 1.0000x reference)
# Trainium2 Bass kernel for nn_Model_26190710571339 (topk_masking).
#
# Model: scores = einsum('bnf,f->bn', feats, w_conv); per-bag sort -> bottom-5
# and top-5 score values -> tiny MLP (10->200->100->1, sigmoid) -> logits, probs.
#
# Sharding: data-parallel over the bag axis; 2 bags per NeuronCore x 8 cores.
# Weights replicated.
#
# v5: the stationary-feats design (one 128-column LDWEIGHTS + 1-column
# matmul per 128 tiles = 8192 PE instructions) was limited by PE
# INSTRUCTION FETCH: the sequencer streams 512 KB of instructions from HBM
# at ~16 KB per ~6.6 us, stalling the PE ~2.8 us per stripe. This version
# flips the matmul orientation so one instruction covers 512 tiles x 256
# features:
#   - feats staged as fp8_e4m3 in DoubleRow layout: rhs [128p=f, 2 k-tiles,
#     512 n] per matmul, lhsT = w pair [128, 2] fp8 (x64, rescaled in the
#     PSUM->SBUF copy). fp8 DoubleRow runs the PE at 2x bf16 rate.
#     512 matmuls + 512 ldweights total -> ~70 KB instruction stream.
#   - scores land in PSUM rows [1, 512] (partition 0), accumulated over the
#     8 chunk-pairs within one 2048-tile window; windows stream as 2 MB
#     cp-quad sub-DMAs with 16 KB contiguous lines on the 2 HWDGE queues.
#   - top/bottom-5 via DVE max8 on the window score rows (top-8 of each
#     2048-tile window, then top-8 of the window candidates per bag —
#     exact containment). Bottom side via a negated copy (ACT engine,
#     Copy activation with scale=-1/64). The last window runs max8
#     per 512-row to shorten the serial tail.
#   - the 10x2 MLP input is assembled directly by two tiny DMAs per bag
#     (no PE transpose); descending order + sign are folded into a host
#     permutation of W1's columns. Quantization rel err ~7e-3 (gate 2e-2).

import numpy as np

B = 16
NTILES = 16384
FSZ = 2048
R = 5
NCORES = 8
BAGS_PER_CORE = B // NCORES  # 2

NWIN = 16            # windows per core
WINN = 2048          # tiles per window
NSUB = 2             # cp-quad sub-DMAs per window
NCP = 8              # chunk pairs (256 features each)
ROWS = WINN // 512   # psum rows per window (4)
WSCALE = 64.0        # w is staged as fp8(64*w); copies rescale by 1/64


def _build_nc(nbags, ntiles, fsz, bufs=7, ncores=NCORES):
    import concourse.mybir as mybir
    import concourse.tile as tile
    from concourse import bacc
    from contextlib import ExitStack

    f32 = mybir.dt.float32
    f8 = mybir.dt.float8e4
    Act = mybir.ActivationFunctionType
    DR = mybir.MatmulPerfMode.DoubleRow

    win_per_bag = NWIN // nbags  # 8

    nc = bacc.Bacc("TRN2", target_bir_lowering=False, debug=False, num_devices=ncores)
    # ft8[w, k, p, (c4, j, n)]: feats8[w*WINN + n, ((k*4+c4)*2+j)*128 + p]
    ft8 = nc.declare_dram_parameter("ft8", [NWIN, NSUB, 128, 4 * 2 * WINN], f8, isOutput=False)
    w8 = nc.declare_dram_parameter("w8", [128, 2, 16], f8, isOutput=False)
    w1t = nc.declare_dram_parameter("w1t", [2 * R, 200], f32, isOutput=False)
    w2ta = nc.declare_dram_parameter("w2ta", [128, 100], f32, isOutput=False)
    w2tb = nc.declare_dram_parameter("w2tb", [72, 100], f32, isOutput=False)
    w3t = nc.declare_dram_parameter("w3t", [100, 1], f32, isOutput=False)
    b1a = nc.declare_dram_parameter("b1a", [128, 1], f32, isOutput=False)
    b1b = nc.declare_dram_parameter("b1b", [72, 1], f32, isOutput=False)
    b2c = nc.declare_dram_parameter("b2c", [100, 1], f32, isOutput=False)
    b3c = nc.declare_dram_parameter("b3c", [1, 1], f32, isOutput=False)
    logits_o = nc.declare_dram_parameter("logits", [1, nbags], f32, isOutput=True)
    probs_o = nc.declare_dram_parameter("probs", [1, nbags], f32, isOutput=True)

    with ExitStack() as ctx:
        tc = ctx.enter_context(tile.TileContext(nc))
        consts = ctx.enter_context(tc.tile_pool(name="consts", bufs=1))

        w8_sb = consts.tile([128, 2, 16], f8)
        nc.sync.dma_start(w8_sb[:], w8[:])

        fpool = ctx.enter_context(tc.tile_pool(name="fpool", bufs=bufs))
        wpool = ctx.enter_context(tc.tile_pool(name="wpool", bufs=4))
        psmain = ctx.enter_context(tc.tile_pool(name="psmain", bufs=4, space="PSUM"))
        tpool = ctx.enter_context(tc.tile_pool(name="tpool", bufs=1))
        hwdge = [nc.sync, nc.scalar]

        # per-bag row-candidate slots: 8 windows x 4 rows x 8
        wt8max = [tpool.tile([1, 256], f32, name=f"wt8max{b}") for b in range(nbags)]
        wt8min = [tpool.tile([1, 256], f32, name=f"wt8min{b}") for b in range(nbags)]
        # fin[b]: [bottom-8 negated | top-8], feeds the mmT column DMAs
        fin = [tpool.tile([1, 16], f32, name=f"fin{b}") for b in range(nbags)]
        mmT = tpool.tile([2 * R, nbags], f32)

        # MLP consts early on the idle SWDGE queue; they have no deps so
        # they never stall it, and the ACT table preload can happen during
        # the stream instead of in the tail.
        w1t_sb = consts.tile([2 * R, 200], f32)
        nc.gpsimd.dma_start(w1t_sb[:], w1t[:])
        w2ta_sb = consts.tile([128, 100], f32)
        nc.gpsimd.dma_start(w2ta_sb[:], w2ta[:])
        w2tb_sb = consts.tile([72, 100], f32)
        nc.gpsimd.dma_start(w2tb_sb[:], w2tb[:])
        w3t_sb = consts.tile([100, 1], f32)
        nc.gpsimd.dma_start(w3t_sb[:], w3t[:])
        b1a_sb = consts.tile([128, 1], f32)
        nc.gpsimd.dma_start(b1a_sb[:], b1a[:])
        b1b_sb = consts.tile([72, 1], f32)
        nc.gpsimd.dma_start(b1b_sb[:], b1b[:])
        b2c_sb = consts.tile([100, 1], f32)
        nc.gpsimd.dma_start(b2c_sb[:], b2c[:])
        b3c_sb = consts.tile([1, 1], f32)
        nc.gpsimd.dma_start(b3c_sb[:], b3c[:])
        actwarm = consts.tile([1, 1], f32)
        nc.scalar.activation(actwarm[:], b3c_sb[:], Act.Sigmoid)

        ndma = 0
        for w in range(NWIN):
            bag = w // win_per_bag
            last_of_bag = (w + 1) % win_per_bag == 0
            # first/last window stream as 4 x 1 MB cp-pair subs (8 KB
            # lines) for fast PE ramp/drain; the rest as 2 x 2 MB cp-quads
            # (16 KB lines) for best DMA engine efficiency.
            if w == 0 or w == NWIN - 1:
                groups = [(g * 2, 2) for g in range(4)]
            else:
                groups = [(0, 4), (4, 4)]
            subs = []
            for cp0, ncp in groups:
                sub = fpool.tile([128, ncp, 2, WINN], f8, name=f"sub{ncp}")
                src_ap = ft8[w, cp0 // 4][
                    :, (cp0 % 4) * 2 * WINN : (cp0 % 4 + ncp) * 2 * WINN
                ]
                hwdge[ndma % 2].dma_start(sub[:], src_ap)
                ndma += 1
                subs.append((sub, cp0, ncp))
            pr = [psmain.tile([128, 512], f32, name="pr") for _ in range(ROWS)]
            for gi, (sub, cp0, ncp) in enumerate(subs):
                last_group = gi == len(subs) - 1
                for s in range(ROWS):
                    for ci in range(ncp):
                        cp = cp0 + ci
                        nc.tensor.matmul(
                            pr[s][0:1, :],
                            lhsT=w8_sb[:, :, cp],
                            rhs=sub[:, ci, :, s * 512 : (s + 1) * 512],
                            start=(cp == 0),
                            stop=(cp == NCP - 1),
                            perf_mode=DR,
                        )
                    if last_group:
                        # per-row: rescale out of PSUM, negate, top-8 both
                        # sides into the bag's candidate slots
                        winrow = wpool.tile([1, 512], f32, name="winrow")
                        negwin = wpool.tile([1, 512], f32, name="negwin")
                        nc.vector.tensor_scalar_mul(
                            winrow[:], pr[s][0:1, :], 1.0 / WSCALE
                        )
                        nc.vector.tensor_scalar_mul(negwin[:], winrow[:], -1.0)
                        o = (w % win_per_bag) * 32 + s * 8
                        nc.vector.max(wt8max[bag][0:1, o : o + 8], winrow[:])
                        nc.vector.max(wt8min[bag][0:1, o : o + 8], negwin[:])
            if w == win_per_bag - 1:
                # bag 0 complete: its finals + mmT column build run during
                # the stream; the tiny DMA goes on the idle SWDGE queue.
                nc.vector.max(fin[0][0:1, 0:8], wt8min[0][0:1, 0:256])
                nc.vector.max(fin[0][0:1, 8:16], wt8max[0][0:1, 0:256])
                nc.gpsimd.dma_start(mmT[0:R, 0:1], fin[0][0:1, 0:R])
                nc.gpsimd.dma_start(mmT[R : 2 * R, 0:1], fin[0][0:1, 8 : 8 + R])

        # ---- bag 1 finals in the tail (bag 0 ran inline above).
        # mmT[j, b] = -(j+1)-th smallest for j<5; (10-j)-th largest for j>=5
        # (host-permuted W1 compensates sign and order)
        b1i = nbags - 1
        nc.vector.max(fin[b1i][0:1, 0:8], wt8min[b1i][0:1, 0:256])
        nc.vector.max(fin[b1i][0:1, 8:16], wt8max[b1i][0:1, 0:256])
        nc.sync.dma_start(mmT[0:R, b1i : b1i + 1], fin[b1i][0:1, 0:R])
        nc.scalar.dma_start(mmT[R : 2 * R, b1i : b1i + 1], fin[b1i][0:1, 8 : 8 + R])

        # ---- MLP (transposed): hT = sigmoid(W @ xT + b), biases per-partition
        psum2 = ctx.enter_context(tc.tile_pool(name="psum2", bufs=1, space="PSUM"))
        h1pa = psum2.tile([128, nbags], f32, name="h1pa")
        h1pb = psum2.tile([72, nbags], f32, name="h1pb")
        nc.tensor.matmul(h1pa[:], lhsT=w1t_sb[:, 0:128], rhs=mmT[:], start=True, stop=True)
        nc.tensor.matmul(h1pb[:], lhsT=w1t_sb[:, 128:200], rhs=mmT[:], start=True, stop=True)
        h1a = tpool.tile([128, nbags], f32)
        h1b = tpool.tile([72, nbags], f32)
        nc.scalar.activation(h1a[:], h1pa[:], Act.Sigmoid, bias=b1a_sb[:], scale=1.0)
        nc.scalar.activation(h1b[:], h1pb[:], Act.Sigmoid, bias=b1b_sb[:], scale=1.0)

        h2p = psum2.tile([100, nbags], f32, name="h2p")
        nc.tensor.matmul(h2p[:], lhsT=w2ta_sb[:], rhs=h1a[:], start=True, stop=False)
        nc.tensor.matmul(h2p[:], lhsT=w2tb_sb[:], rhs=h1b[:], start=False, stop=True)
        h2 = tpool.tile([100, nbags], f32)
        nc.scalar.activation(h2[:], h2p[:], Act.Sigmoid, bias=b2c_sb[:], scale=1.0)

        lp = psum2.tile([1, nbags], f32, name="lp")
        nc.tensor.matmul(lp[:], lhsT=w3t_sb[:], rhs=h2[:], start=True, stop=True)
        lsb = tpool.tile([1, nbags], f32)
        nc.vector.tensor_scalar_add(lsb[:], lp[:], b3c_sb[:])
        psb = tpool.tile([1, nbags], f32)
        nc.scalar.activation(psb[:], lsb[:], Act.Sigmoid)

        nc.sync.dma_start(logits_o[:], lsb[:])
        nc.scalar.dma_start(probs_o[:], psb[:])

    nc.finalize()
    return nc


def _make_in_maps(inputs, nbags, ntiles, fsz, ncores):
    import ml_dtypes

    feats = np.asarray(inputs["feats"], dtype=np.float32)
    w_conv = np.asarray(inputs["w_conv"], dtype=np.float32)
    W1 = np.asarray(inputs["W1"], dtype=np.float32)
    b1 = np.asarray(inputs["b1"], dtype=np.float32)
    W2 = np.asarray(inputs["W2"], dtype=np.float32)
    b2 = np.asarray(inputs["b2"], dtype=np.float32)
    W3 = np.asarray(inputs["W3"], dtype=np.float32)
    b3 = np.asarray(inputs["b3"], dtype=np.float32)

    # Kernel produces mmT[j, b] = -(bottom-(j+1)) for j<5 and
    # top-(j-4)-largest (descending) for j>=5; reference minmax is bottom-5
    # ascending then top-5 ascending. Fold into W1's columns.
    W1_hw = np.empty_like(W1)
    W1_hw[:, 0:R] = -W1[:, 0:R]
    W1_hw[:, R : 2 * R] = W1[:, 2 * R - 1 : R - 1 : -1]

    # w8[p, j, cp] = fp8(WSCALE * w[cp*256 + j*128 + p]), cp slots padded to
    # 16 so the DoubleRow ldweights k-tile step is 16 elements (ISA rule)
    w8v = (WSCALE * w_conv).reshape(NCP, 2, 128).transpose(2, 1, 0)
    w8 = np.zeros((128, 2, 16), dtype=np.float32)
    w8[:, :, :NCP] = w8v
    w8 = np.ascontiguousarray(w8).astype(ml_dtypes.float8_e4m3)

    base = {
        "w8": w8,
        "w1t": np.ascontiguousarray(W1_hw.T),
        "w2ta": np.ascontiguousarray(W2.T[:128]),
        "w2tb": np.ascontiguousarray(W2.T[128:]),
        "w3t": np.ascontiguousarray(W3.T),
        "b1a": np.ascontiguousarray(b1[:128].reshape(128, 1)),
        "b1b": np.ascontiguousarray(b1[128:].reshape(72, 1)),
        "b2c": np.ascontiguousarray(b2.reshape(100, 1)),
        "b3c": np.ascontiguousarray(b3.reshape(1, 1)),
    }
    in_maps = []
    for cid in range(ncores):
        shard = feats[cid * nbags : (cid + 1) * nbags].reshape(nbags * ntiles, fsz)
        q = shard.astype(ml_dtypes.float8_e4m3)
        # [w, n, k, c4, j, p] -> [w, k, p, c4, j, n]
        a = q.reshape(NWIN, WINN, NSUB, 4, 2, 128).transpose(0, 2, 5, 3, 4, 1)
        a = np.ascontiguousarray(a).reshape(NWIN, NSUB, 128, 4 * 2 * WINN)
        in_maps.append({**base, "ft8": a})
    return in_maps


def _run(inputs, trace=False, **spmd_kwargs):
    from concourse.bass_utils import run_bass_kernel_spmd

    nc = _build_nc(BAGS_PER_CORE, NTILES, FSZ)
    in_maps = _make_in_maps(inputs, BAGS_PER_CORE, NTILES, FSZ, NCORES)
    res = run_bass_kernel_spmd(
        nc, in_maps, list(range(NCORES)), trace=trace, **spmd_kwargs
    )
    logits = np.concatenate(
        [res.results[c]["logits"].reshape(BAGS_PER_CORE, 1) for c in range(NCORES)],
        axis=0,
    )
    probs = np.concatenate(
        [res.results[c]["probs"].reshape(BAGS_PER_CORE, 1) for c in range(NCORES)],
        axis=0,
    )
    return (logits, probs), res


def kernel(**inputs):
    out, _ = _run(inputs, trace=False)
    return out


# revision 19
# speedup vs baseline: 1.1199x; 1.1199x over previous
# Trainium2 Bass kernel for nn_Model_26190710571339 (topk_masking).
#
# Model: scores = einsum('bnf,f->bn', feats, w_conv); per-bag sort -> bottom-5
# and top-5 score values -> tiny MLP (10->200->100->1, sigmoid) -> logits, probs.
#
# Sharding: data-parallel over the bag axis; 2 bags per NeuronCore x 8 cores.
# Weights replicated.
#
# v5: the stationary-feats design (one 128-column LDWEIGHTS + 1-column
# matmul per 128 tiles = 8192 PE instructions) was limited by PE
# INSTRUCTION FETCH: the sequencer streams 512 KB of instructions from HBM
# at ~16 KB per ~6.6 us, stalling the PE ~2.8 us per stripe. This version
# flips the matmul orientation so one instruction covers 512 tiles x 256
# features:
#   - feats staged as fp8_e4m3 in DoubleRow layout: rhs [128p=f, 2 k-tiles,
#     512 n] per matmul, lhsT = w pair [128, 2] fp8 (x64, rescaled in the
#     PSUM->SBUF copy). fp8 DoubleRow runs the PE at 2x bf16 rate.
#     512 matmuls + 512 ldweights total -> ~70 KB instruction stream.
#   - scores land in PSUM rows [1, 512] (partition 0), accumulated over the
#     8 chunk-pairs within one 2048-tile window; windows stream as 2 MB
#     cp-quad sub-DMAs with 16 KB contiguous lines on the 2 HWDGE queues.
#   - top/bottom-5 via DVE max8 on the window score rows (top-8 of each
#     2048-tile window, then top-8 of the window candidates per bag —
#     exact containment). Bottom side via a negated copy (ACT engine,
#     Copy activation with scale=-1/64). The last window runs max8
#     per 512-row to shorten the serial tail.
#   - the 10x2 MLP input is assembled directly by two tiny DMAs per bag
#     (no PE transpose); descending order + sign are folded into a host
#     permutation of W1's columns. Quantization rel err ~7e-3 (gate 2e-2).

import numpy as np

B = 16
NTILES = 16384
FSZ = 2048
R = 5
NCORES = 8
BAGS_PER_CORE = B // NCORES  # 2

NWIN = 16            # windows per core
WINN = 2048          # tiles per window
NSUB = 2             # cp-quad sub-DMAs per window
NCP = 8              # chunk pairs (256 features each)
ROWS = WINN // 512   # psum rows per window (4)
WSCALE = 64.0        # w is staged as fp8(64*w); copies rescale by 1/64


def _build_nc(nbags, ntiles, fsz, bufs=6, ncores=NCORES):
    import concourse.mybir as mybir
    import concourse.tile as tile
    from concourse import bacc
    from contextlib import ExitStack

    f32 = mybir.dt.float32
    f8 = mybir.dt.float8e4
    Act = mybir.ActivationFunctionType
    DR = mybir.MatmulPerfMode.DoubleRow

    win_per_bag = NWIN // nbags  # 8

    nc = bacc.Bacc("TRN2", target_bir_lowering=False, debug=False, num_devices=ncores)
    # ft8[w, k, p, (c4, j, n)]: feats8[w*WINN + n, ((k*4+c4)*2+j)*128 + p]
    ft8 = nc.declare_dram_parameter("ft8", [NWIN, NSUB, 128, 4 * 2 * WINN], f8, isOutput=False)
    w8 = nc.declare_dram_parameter("w8", [128, 2, 16], f8, isOutput=False)
    w1t = nc.declare_dram_parameter("w1t", [2 * R, 200], f32, isOutput=False)
    w2ta = nc.declare_dram_parameter("w2ta", [128, 100], f32, isOutput=False)
    w2tb = nc.declare_dram_parameter("w2tb", [72, 100], f32, isOutput=False)
    w3t = nc.declare_dram_parameter("w3t", [100, 1], f32, isOutput=False)
    b1a = nc.declare_dram_parameter("b1a", [128, 1], f32, isOutput=False)
    b1b = nc.declare_dram_parameter("b1b", [72, 1], f32, isOutput=False)
    b2c = nc.declare_dram_parameter("b2c", [100, 1], f32, isOutput=False)
    b3c = nc.declare_dram_parameter("b3c", [1, 1], f32, isOutput=False)
    logits_o = nc.declare_dram_parameter("logits", [1, nbags], f32, isOutput=True)
    probs_o = nc.declare_dram_parameter("probs", [1, nbags], f32, isOutput=True)

    with ExitStack() as ctx:
        tc = ctx.enter_context(tile.TileContext(nc))
        consts = ctx.enter_context(tc.tile_pool(name="consts", bufs=1))

        w8_sb = consts.tile([128, 2, 16], f8)
        nc.sync.dma_start(w8_sb[:], w8[:])

        fpool = ctx.enter_context(tc.tile_pool(name="fpool", bufs=bufs))
        wpool = ctx.enter_context(tc.tile_pool(name="wpool", bufs=2))
        psmain = ctx.enter_context(tc.tile_pool(name="psmain", bufs=4, space="PSUM"))
        tpool = ctx.enter_context(tc.tile_pool(name="tpool", bufs=1))
        hwdge = [nc.sync, nc.scalar]

        # per-bag window-candidate slots: 8 windows x 8
        wt8max = [tpool.tile([1, 64], f32, name=f"wt8max{b}") for b in range(nbags)]
        wt8min = [tpool.tile([1, 64], f32, name=f"wt8min{b}") for b in range(nbags)]
        # fin[b]: [bottom-8 negated | top-8], feeds the mmT column DMAs
        fin = [tpool.tile([1, 16], f32, name=f"fin{b}") for b in range(nbags)]
        mmT = tpool.tile([2 * R, nbags], f32)

        # MLP consts early on the idle SWDGE queue; they have no deps so
        # they never stall it, and the ACT table preload can happen during
        # the stream instead of in the tail.
        w1t_sb = consts.tile([2 * R, 200], f32)
        nc.gpsimd.dma_start(w1t_sb[:], w1t[:])
        w2ta_sb = consts.tile([128, 100], f32)
        nc.gpsimd.dma_start(w2ta_sb[:], w2ta[:])
        w2tb_sb = consts.tile([72, 100], f32)
        nc.gpsimd.dma_start(w2tb_sb[:], w2tb[:])
        w3t_sb = consts.tile([100, 1], f32)
        nc.gpsimd.dma_start(w3t_sb[:], w3t[:])
        b1a_sb = consts.tile([128, 1], f32)
        nc.gpsimd.dma_start(b1a_sb[:], b1a[:])
        b1b_sb = consts.tile([72, 1], f32)
        nc.gpsimd.dma_start(b1b_sb[:], b1b[:])
        b2c_sb = consts.tile([100, 1], f32)
        nc.gpsimd.dma_start(b2c_sb[:], b2c[:])
        b3c_sb = consts.tile([1, 1], f32)
        nc.gpsimd.dma_start(b3c_sb[:], b3c[:])
        actwarm = consts.tile([1, 1], f32)
        nc.scalar.activation(actwarm[:], b3c_sb[:], Act.Sigmoid)

        ndma = 0
        for w in range(NWIN):
            bag = w // win_per_bag
            last_of_bag = (w + 1) % win_per_bag == 0
            # first/last window stream as 4 x 1 MB cp-pair subs (8 KB
            # lines) for fast PE ramp/drain; the rest as 2 x 2 MB cp-quads
            # (16 KB lines) for best DMA engine efficiency.
            if w == 0 or w == NWIN - 1:
                groups = [(g * 2, 2) for g in range(4)]
            else:
                groups = [(0, 4), (4, 4)]
            subs = []
            for cp0, ncp in groups:
                sub = fpool.tile([128, ncp, 2, WINN], f8, name=f"sub{ncp}")
                src_ap = ft8[w, cp0 // 4][
                    :, (cp0 % 4) * 2 * WINN : (cp0 % 4 + ncp) * 2 * WINN
                ]
                hwdge[ndma % 2].dma_start(sub[:], src_ap)
                ndma += 1
                subs.append((sub, cp0, ncp))
            pr = [psmain.tile([128, 512], f32, name="pr") for _ in range(ROWS)]
            winrow = wpool.tile([1, WINN], f32, name="winrow")
            negwin = wpool.tile([1, WINN], f32, name="negwin")
            for gi, (sub, cp0, ncp) in enumerate(subs):
                last_group = gi == len(subs) - 1
                for s in range(ROWS):
                    for ci in range(ncp):
                        cp = cp0 + ci
                        nc.tensor.matmul(
                            pr[s][0:1, :],
                            lhsT=w8_sb[:, :, cp],
                            rhs=sub[:, ci, :, s * 512 : (s + 1) * 512],
                            start=(cp == 0),
                            stop=(cp == NCP - 1),
                            perf_mode=DR,
                        )
                    if last_group:
                        nc.vector.tensor_scalar_mul(
                            winrow[0:1, s * 512 : (s + 1) * 512],
                            pr[s][0:1, :],
                            1.0 / WSCALE,
                        )
            # window complete: negate + top-8 of both sides (max8 has a
            # ~660 ns floor, so one call per window, not per row)
            nc.vector.tensor_scalar_mul(negwin[:], winrow[:], -1.0)
            o = (w % win_per_bag) * 8
            nc.vector.max(wt8max[bag][0:1, o : o + 8], winrow[:])
            nc.vector.max(wt8min[bag][0:1, o : o + 8], negwin[:])
            if w == win_per_bag - 1:
                # bag 0 complete: its finals + mmT column build run during
                # the stream; the tiny DMA goes on the idle SWDGE queue.
                nc.vector.max(fin[0][0:1, 0:8], wt8min[0][0:1, 0:64])
                nc.vector.max(fin[0][0:1, 8:16], wt8max[0][0:1, 0:64])
                nc.gpsimd.dma_start(mmT[0:R, 0:1], fin[0][0:1, 0:R])
                nc.gpsimd.dma_start(mmT[R : 2 * R, 0:1], fin[0][0:1, 8 : 8 + R])

        # ---- bag 1 finals in the tail (bag 0 ran inline above).
        # mmT[j, b] = -(j+1)-th smallest for j<5; (10-j)-th largest for j>=5
        # (host-permuted W1 compensates sign and order)
        b1i = nbags - 1
        nc.vector.max(fin[b1i][0:1, 0:8], wt8min[b1i][0:1, 0:64])
        nc.vector.max(fin[b1i][0:1, 8:16], wt8max[b1i][0:1, 0:64])
        nc.sync.dma_start(mmT[0:R, b1i : b1i + 1], fin[b1i][0:1, 0:R])
        nc.scalar.dma_start(mmT[R : 2 * R, b1i : b1i + 1], fin[b1i][0:1, 8 : 8 + R])

        # ---- MLP (transposed): hT = sigmoid(W @ xT + b), biases per-partition
        psum2 = ctx.enter_context(tc.tile_pool(name="psum2", bufs=1, space="PSUM"))
        h1pa = psum2.tile([128, nbags], f32, name="h1pa")
        h1pb = psum2.tile([72, nbags], f32, name="h1pb")
        nc.tensor.matmul(h1pa[:], lhsT=w1t_sb[:, 0:128], rhs=mmT[:], start=True, stop=True)
        nc.tensor.matmul(h1pb[:], lhsT=w1t_sb[:, 128:200], rhs=mmT[:], start=True, stop=True)
        h1a = tpool.tile([128, nbags], f32)
        h1b = tpool.tile([72, nbags], f32)
        nc.scalar.activation(h1a[:], h1pa[:], Act.Sigmoid, bias=b1a_sb[:], scale=1.0)
        nc.scalar.activation(h1b[:], h1pb[:], Act.Sigmoid, bias=b1b_sb[:], scale=1.0)

        h2p = psum2.tile([100, nbags], f32, name="h2p")
        nc.tensor.matmul(h2p[:], lhsT=w2ta_sb[:], rhs=h1a[:], start=True, stop=False)
        nc.tensor.matmul(h2p[:], lhsT=w2tb_sb[:], rhs=h1b[:], start=False, stop=True)
        h2 = tpool.tile([100, nbags], f32)
        nc.scalar.activation(h2[:], h2p[:], Act.Sigmoid, bias=b2c_sb[:], scale=1.0)

        lp = psum2.tile([1, nbags], f32, name="lp")
        nc.tensor.matmul(lp[:], lhsT=w3t_sb[:], rhs=h2[:], start=True, stop=True)
        lsb = tpool.tile([1, nbags], f32)
        nc.vector.tensor_scalar_add(lsb[:], lp[:], b3c_sb[:])
        psb = tpool.tile([1, nbags], f32)
        nc.scalar.activation(psb[:], lsb[:], Act.Sigmoid)

        nc.sync.dma_start(logits_o[:], lsb[:])
        nc.scalar.dma_start(probs_o[:], psb[:])

    nc.finalize()
    return nc


def _make_in_maps(inputs, nbags, ntiles, fsz, ncores):
    import ml_dtypes

    feats = np.asarray(inputs["feats"], dtype=np.float32)
    w_conv = np.asarray(inputs["w_conv"], dtype=np.float32)
    W1 = np.asarray(inputs["W1"], dtype=np.float32)
    b1 = np.asarray(inputs["b1"], dtype=np.float32)
    W2 = np.asarray(inputs["W2"], dtype=np.float32)
    b2 = np.asarray(inputs["b2"], dtype=np.float32)
    W3 = np.asarray(inputs["W3"], dtype=np.float32)
    b3 = np.asarray(inputs["b3"], dtype=np.float32)

    # Kernel produces mmT[j, b] = -(bottom-(j+1)) for j<5 and
    # top-(j-4)-largest (descending) for j>=5; reference minmax is bottom-5
    # ascending then top-5 ascending. Fold into W1's columns.
    W1_hw = np.empty_like(W1)
    W1_hw[:, 0:R] = -W1[:, 0:R]
    W1_hw[:, R : 2 * R] = W1[:, 2 * R - 1 : R - 1 : -1]

    # w8[p, j, cp] = fp8(WSCALE * w[cp*256 + j*128 + p]), cp slots padded to
    # 16 so the DoubleRow ldweights k-tile step is 16 elements (ISA rule)
    w8v = (WSCALE * w_conv).reshape(NCP, 2, 128).transpose(2, 1, 0)
    w8 = np.zeros((128, 2, 16), dtype=np.float32)
    w8[:, :, :NCP] = w8v
    w8 = np.ascontiguousarray(w8).astype(ml_dtypes.float8_e4m3)

    base = {
        "w8": w8,
        "w1t": np.ascontiguousarray(W1_hw.T),
        "w2ta": np.ascontiguousarray(W2.T[:128]),
        "w2tb": np.ascontiguousarray(W2.T[128:]),
        "w3t": np.ascontiguousarray(W3.T),
        "b1a": np.ascontiguousarray(b1[:128].reshape(128, 1)),
        "b1b": np.ascontiguousarray(b1[128:].reshape(72, 1)),
        "b2c": np.ascontiguousarray(b2.reshape(100, 1)),
        "b3c": np.ascontiguousarray(b3.reshape(1, 1)),
    }
    in_maps = []
    for cid in range(ncores):
        shard = feats[cid * nbags : (cid + 1) * nbags].reshape(nbags * ntiles, fsz)
        q = shard.astype(ml_dtypes.float8_e4m3)
        # [w, n, k, c4, j, p] -> [w, k, p, c4, j, n]
        a = q.reshape(NWIN, WINN, NSUB, 4, 2, 128).transpose(0, 2, 5, 3, 4, 1)
        a = np.ascontiguousarray(a).reshape(NWIN, NSUB, 128, 4 * 2 * WINN)
        in_maps.append({**base, "ft8": a})
    return in_maps


def _run(inputs, trace=False, **spmd_kwargs):
    from concourse.bass_utils import run_bass_kernel_spmd

    nc = _build_nc(BAGS_PER_CORE, NTILES, FSZ)
    in_maps = _make_in_maps(inputs, BAGS_PER_CORE, NTILES, FSZ, NCORES)
    res = run_bass_kernel_spmd(
        nc, in_maps, list(range(NCORES)), trace=trace, **spmd_kwargs
    )
    logits = np.concatenate(
        [res.results[c]["logits"].reshape(BAGS_PER_CORE, 1) for c in range(NCORES)],
        axis=0,
    )
    probs = np.concatenate(
        [res.results[c]["probs"].reshape(BAGS_PER_CORE, 1) for c in range(NCORES)],
        axis=0,
    )
    return (logits, probs), res


def kernel(**inputs):
    out, _ = _run(inputs, trace=False)
    return out


# revision 20
# speedup vs baseline: 1.1808x; 1.0543x over previous
# Trainium2 Bass kernel for nn_Model_26190710571339 (topk_masking).
#
# Model: scores = einsum('bnf,f->bn', feats, w_conv); per-bag sort -> bottom-5
# and top-5 score values -> tiny MLP (10->200->100->1, sigmoid) -> logits, probs.
#
# Sharding: data-parallel over the bag axis; 2 bags per NeuronCore x 8 cores.
# Weights replicated.
#
# v5: the stationary-feats design (one 128-column LDWEIGHTS + 1-column
# matmul per 128 tiles = 8192 PE instructions) was limited by PE
# INSTRUCTION FETCH: the sequencer streams 512 KB of instructions from HBM
# at ~16 KB per ~6.6 us, stalling the PE ~2.8 us per stripe. This version
# flips the matmul orientation so one instruction covers 512 tiles x 256
# features:
#   - feats staged as fp8_e4m3 in DoubleRow layout: rhs [128p=f, 2 k-tiles,
#     512 n] per matmul, lhsT = w pair [128, 2] fp8 (x64, rescaled in the
#     PSUM->SBUF copy). fp8 DoubleRow runs the PE at 2x bf16 rate.
#     512 matmuls + 512 ldweights total -> ~70 KB instruction stream.
#   - scores land in PSUM rows [1, 512] (partition 0), accumulated over the
#     8 chunk-pairs within one 2048-tile window; windows stream as 2 MB
#     cp-quad sub-DMAs with 16 KB contiguous lines on the 2 HWDGE queues.
#   - top/bottom-5 via DVE max8 on the window score rows (top-8 of each
#     2048-tile window, then top-8 of the window candidates per bag —
#     exact containment). Bottom side via a negated copy (ACT engine,
#     Copy activation with scale=-1/64). The last window runs max8
#     per 512-row to shorten the serial tail.
#   - the 10x2 MLP input is assembled directly by two tiny DMAs per bag
#     (no PE transpose); descending order + sign are folded into a host
#     permutation of W1's columns. Quantization rel err ~7e-3 (gate 2e-2).

import numpy as np

B = 16
NTILES = 16384
FSZ = 2048
R = 5
NCORES = 8
BAGS_PER_CORE = B // NCORES  # 2

NWIN = 16            # windows per core
WINN = 2048          # tiles per window
NSUB = 2             # cp-quad sub-DMAs per window
NCP = 8              # chunk pairs (256 features each)
ROWS = WINN // 512   # psum rows per window (4)
WSCALE = 64.0        # w is staged as fp8(64*w); copies rescale by 1/64


def _build_nc(nbags, ntiles, fsz, bufs=8, ncores=NCORES):
    import concourse.mybir as mybir
    import concourse.tile as tile
    from concourse import bacc
    from contextlib import ExitStack

    f32 = mybir.dt.float32
    f8 = mybir.dt.float8e4
    Act = mybir.ActivationFunctionType
    DR = mybir.MatmulPerfMode.DoubleRow

    win_per_bag = NWIN // nbags  # 8

    nc = bacc.Bacc("TRN2", target_bir_lowering=False, debug=False, num_devices=ncores)
    # ft8[w, k, p, (c4, j, n)]: feats8[w*WINN + n, ((k*4+c4)*2+j)*128 + p]
    ft8 = nc.declare_dram_parameter("ft8", [NWIN, NSUB, 128, 4 * 2 * WINN], f8, isOutput=False)
    w8 = nc.declare_dram_parameter("w8", [128, 2, 16], f8, isOutput=False)
    w1t = nc.declare_dram_parameter("w1t", [2 * R, 200], f32, isOutput=False)
    w2ta = nc.declare_dram_parameter("w2ta", [128, 100], f32, isOutput=False)
    w2tb = nc.declare_dram_parameter("w2tb", [72, 100], f32, isOutput=False)
    w3t = nc.declare_dram_parameter("w3t", [100, 1], f32, isOutput=False)
    b1a = nc.declare_dram_parameter("b1a", [128, 1], f32, isOutput=False)
    b1b = nc.declare_dram_parameter("b1b", [72, 1], f32, isOutput=False)
    b2c = nc.declare_dram_parameter("b2c", [100, 1], f32, isOutput=False)
    b3c = nc.declare_dram_parameter("b3c", [1, 1], f32, isOutput=False)
    logits_o = nc.declare_dram_parameter("logits", [1, nbags], f32, isOutput=True)
    probs_o = nc.declare_dram_parameter("probs", [1, nbags], f32, isOutput=True)

    with ExitStack() as ctx:
        tc = ctx.enter_context(tile.TileContext(nc))
        consts = ctx.enter_context(tc.tile_pool(name="consts", bufs=1))

        w8_sb = consts.tile([128, 2, 16], f8)
        nc.sync.dma_start(w8_sb[:], w8[:])

        fpool = ctx.enter_context(tc.tile_pool(name="fpool", bufs=bufs))
        wpool = ctx.enter_context(tc.tile_pool(name="wpool", bufs=2))
        psmain = ctx.enter_context(tc.tile_pool(name="psmain", bufs=4, space="PSUM"))
        tpool = ctx.enter_context(tc.tile_pool(name="tpool", bufs=1))
        hwdge = [nc.sync, nc.scalar]

        # per-bag window-candidate rows: 7 windows x 8 + last window 4 rows x 8
        wt8max = [tpool.tile([1, 96], f32, name=f"wt8max{b}") for b in range(nbags)]
        wt8min = [tpool.tile([1, 96], f32, name=f"wt8min{b}") for b in range(nbags)]

        # MLP consts early on the idle SWDGE queue (no deps, never stalls);
        # sigmoid-set ACT table preloads during the stream, not in the tail.
        w1t_sb = consts.tile([2 * R, 200], f32)
        nc.gpsimd.dma_start(w1t_sb[:], w1t[:])
        w2ta_sb = consts.tile([128, 100], f32)
        nc.gpsimd.dma_start(w2ta_sb[:], w2ta[:])
        w2tb_sb = consts.tile([72, 100], f32)
        nc.gpsimd.dma_start(w2tb_sb[:], w2tb[:])
        w3t_sb = consts.tile([100, 1], f32)
        nc.gpsimd.dma_start(w3t_sb[:], w3t[:])
        b1a_sb = consts.tile([128, 1], f32)
        nc.gpsimd.dma_start(b1a_sb[:], b1a[:])
        b1b_sb = consts.tile([72, 1], f32)
        nc.gpsimd.dma_start(b1b_sb[:], b1b[:])
        b2c_sb = consts.tile([100, 1], f32)
        nc.gpsimd.dma_start(b2c_sb[:], b2c[:])
        b3c_sb = consts.tile([1, 1], f32)
        nc.gpsimd.dma_start(b3c_sb[:], b3c[:])
        actwarm = consts.tile([1, 1], f32)
        nc.scalar.activation(actwarm[:], b3c_sb[:], Act.Sigmoid)

        # fin[b]: [bottom-8 negated | top-8] -> two mmT column DMAs per bag
        fin = [tpool.tile([1, 16], f32, name=f"fin{b}") for b in range(nbags)]
        mmT = tpool.tile([2 * R, nbags], f32)

        ndma = 0
        for w in range(NWIN):
            bag = w // win_per_bag
            last_of_bag = (w + 1) % win_per_bag == 0
            subs = []
            for k in range(NSUB):
                sub = fpool.tile([128, 4, 2, WINN], f8, name="sub")
                hwdge[ndma % 2].dma_start(sub[:], ft8[w, k])
                ndma += 1
                subs.append(sub)
            winrow = wpool.tile([1, WINN], f32, name="winrow")
            negwin = wpool.tile([1, WINN], f32, name="negwin")
            pr = [psmain.tile([128, 512], f32, name="pr") for _ in range(ROWS)]
            for k in range(NSUB):
                for s in range(ROWS):
                    for c4 in range(4):
                        cp = k * 4 + c4
                        nc.tensor.matmul(
                            pr[s][0:1, :],
                            lhsT=w8_sb[:, :, cp],
                            rhs=subs[k][:, c4, :, s * 512 : (s + 1) * 512],
                            start=(cp == 0),
                            stop=(cp == NCP - 1),
                            perf_mode=DR,
                        )
                    if k == NSUB - 1:
                        rsl = slice(s * 512, (s + 1) * 512)
                        nc.vector.tensor_scalar_mul(
                            winrow[0:1, rsl], pr[s][0:1, :], 1.0 / WSCALE
                        )
                        nc.scalar.activation(
                            negwin[0:1, rsl], pr[s][0:1, :], Act.Copy,
                            scale=-1.0 / WSCALE,
                        )
                        if w == NWIN - 1:
                            # last window: per-row candidates to shorten the
                            # serial tail after the final sub-DMA lands
                            o = 56 + s * 8
                            nc.vector.max(wt8max[bag][0:1, o : o + 8], winrow[0:1, rsl])
                            nc.vector.max(wt8min[bag][0:1, o : o + 8], negwin[0:1, rsl])
            if w != NWIN - 1:
                o = (w % win_per_bag) * 8
                nc.vector.max(wt8max[bag][0:1, o : o + 8], winrow[:])
                nc.vector.max(wt8min[bag][0:1, o : o + 8], negwin[:])
            if w == win_per_bag - 1:
                # bag 0 complete: finals + its mmT columns during the stream
                # (tiny DMAs on the otherwise-idle SWDGE queue)
                nc.vector.max(fin[0][0:1, 8:16], wt8max[0][0:1, 0:64])
                nc.vector.max(fin[0][0:1, 0:8], wt8min[0][0:1, 0:64])
                nc.gpsimd.dma_start(mmT[0:R, 0:1], fin[0][0:1, 0:R])
                nc.gpsimd.dma_start(mmT[R : 2 * R, 0:1], fin[0][0:1, 8 : 8 + R])

        # ---- bag-1 finals in the tail (bag 0 ran inline above).
        # mmT[j, b] = -(j+1)-th smallest for j<5; (10-j)-th largest for j>=5
        # (host-permuted W1 compensates sign and order)
        b1i = nbags - 1
        nc.vector.max(fin[b1i][0:1, 8:16], wt8max[b1i][0:1, 0:88])
        nc.vector.max(fin[b1i][0:1, 0:8], wt8min[b1i][0:1, 0:88])
        nc.sync.dma_start(mmT[0:R, b1i : b1i + 1], fin[b1i][0:1, 0:R])
        nc.scalar.dma_start(mmT[R : 2 * R, b1i : b1i + 1], fin[b1i][0:1, 8 : 8 + R])

        # ---- MLP (transposed): hT = sigmoid(W @ xT + b), biases per-partition
        psum2 = ctx.enter_context(tc.tile_pool(name="psum2", bufs=1, space="PSUM"))
        h1pa = psum2.tile([128, nbags], f32, name="h1pa")
        h1pb = psum2.tile([72, nbags], f32, name="h1pb")
        nc.tensor.matmul(h1pa[:], lhsT=w1t_sb[:, 0:128], rhs=mmT[:], start=True, stop=True)
        nc.tensor.matmul(h1pb[:], lhsT=w1t_sb[:, 128:200], rhs=mmT[:], start=True, stop=True)
        h1a = tpool.tile([128, nbags], f32)
        h1b = tpool.tile([72, nbags], f32)
        nc.scalar.activation(h1a[:], h1pa[:], Act.Sigmoid, bias=b1a_sb[:], scale=1.0)
        nc.scalar.activation(h1b[:], h1pb[:], Act.Sigmoid, bias=b1b_sb[:], scale=1.0)

        h2p = psum2.tile([100, nbags], f32, name="h2p")
        nc.tensor.matmul(h2p[:], lhsT=w2ta_sb[:], rhs=h1a[:], start=True, stop=False)
        nc.tensor.matmul(h2p[:], lhsT=w2tb_sb[:], rhs=h1b[:], start=False, stop=True)
        h2 = tpool.tile([100, nbags], f32)
        nc.scalar.activation(h2[:], h2p[:], Act.Sigmoid, bias=b2c_sb[:], scale=1.0)

        lp = psum2.tile([1, nbags], f32, name="lp")
        nc.tensor.matmul(lp[:], lhsT=w3t_sb[:], rhs=h2[:], start=True, stop=True)
        lsb = tpool.tile([1, nbags], f32)
        nc.vector.tensor_scalar_add(lsb[:], lp[:], b3c_sb[:])
        psb = tpool.tile([1, nbags], f32)
        nc.scalar.activation(psb[:], lsb[:], Act.Sigmoid)

        nc.sync.dma_start(logits_o[:], lsb[:])
        nc.scalar.dma_start(probs_o[:], psb[:])

    nc.finalize()
    return nc


def _make_in_maps(inputs, nbags, ntiles, fsz, ncores):
    import ml_dtypes

    feats = np.asarray(inputs["feats"], dtype=np.float32)
    w_conv = np.asarray(inputs["w_conv"], dtype=np.float32)
    W1 = np.asarray(inputs["W1"], dtype=np.float32)
    b1 = np.asarray(inputs["b1"], dtype=np.float32)
    W2 = np.asarray(inputs["W2"], dtype=np.float32)
    b2 = np.asarray(inputs["b2"], dtype=np.float32)
    W3 = np.asarray(inputs["W3"], dtype=np.float32)
    b3 = np.asarray(inputs["b3"], dtype=np.float32)

    # Kernel produces mmT[j, b] = -(bottom-(j+1)) for j<5 and
    # top-(j-4)-largest (descending) for j>=5; reference minmax is bottom-5
    # ascending then top-5 ascending. Fold into W1's columns.
    W1_hw = np.empty_like(W1)
    W1_hw[:, 0:R] = -W1[:, 0:R]
    W1_hw[:, R : 2 * R] = W1[:, 2 * R - 1 : R - 1 : -1]

    # w8[p, j, cp] = fp8(WSCALE * w[cp*256 + j*128 + p]), cp slots padded to
    # 16 so the DoubleRow ldweights k-tile step is 16 elements (ISA rule)
    w8v = (WSCALE * w_conv).reshape(NCP, 2, 128).transpose(2, 1, 0)
    w8 = np.zeros((128, 2, 16), dtype=np.float32)
    w8[:, :, :NCP] = w8v
    w8 = np.ascontiguousarray(w8).astype(ml_dtypes.float8_e4m3)

    base = {
        "w8": w8,
        "w1t": np.ascontiguousarray(W1_hw.T),
        "w2ta": np.ascontiguousarray(W2.T[:128]),
        "w2tb": np.ascontiguousarray(W2.T[128:]),
        "w3t": np.ascontiguousarray(W3.T),
        "b1a": np.ascontiguousarray(b1[:128].reshape(128, 1)),
        "b1b": np.ascontiguousarray(b1[128:].reshape(72, 1)),
        "b2c": np.ascontiguousarray(b2.reshape(100, 1)),
        "b3c": np.ascontiguousarray(b3.reshape(1, 1)),
    }
    in_maps = []
    for cid in range(ncores):
        shard = feats[cid * nbags : (cid + 1) * nbags].reshape(nbags * ntiles, fsz)
        q = shard.astype(ml_dtypes.float8_e4m3)
        # [w, n, k, c4, j, p] -> [w, k, p, c4, j, n]
        a = q.reshape(NWIN, WINN, NSUB, 4, 2, 128).transpose(0, 2, 5, 3, 4, 1)
        a = np.ascontiguousarray(a).reshape(NWIN, NSUB, 128, 4 * 2 * WINN)
        in_maps.append({**base, "ft8": a})
    return in_maps


def _run(inputs, trace=False, **spmd_kwargs):
    from concourse.bass_utils import run_bass_kernel_spmd

    nc = _build_nc(BAGS_PER_CORE, NTILES, FSZ)
    in_maps = _make_in_maps(inputs, BAGS_PER_CORE, NTILES, FSZ, NCORES)
    res = run_bass_kernel_spmd(
        nc, in_maps, list(range(NCORES)), trace=trace, **spmd_kwargs
    )
    logits = np.concatenate(
        [res.results[c]["logits"].reshape(BAGS_PER_CORE, 1) for c in range(NCORES)],
        axis=0,
    )
    probs = np.concatenate(
        [res.results[c]["probs"].reshape(BAGS_PER_CORE, 1) for c in range(NCORES)],
        axis=0,
    )
    return (logits, probs), res


def kernel(**inputs):
    out, _ = _run(inputs, trace=False)
    return out
